# revision 11
# baseline (speedup 1.0000x reference)
"""Trainium2 Bass kernel for DiverseSiblingsSearch (per-beam top-k + sibling
penalty + cross-beam top-k).

Contract: kernel(**inputs) takes the FULL inputs (lprobs [128,5,50257] f32,
scores [128,5,10] f32, step scalar) and returns the FULL outputs
(final_scores [128,10] f32, final_indices [128,10] i32, final_beams [128,10] i32).

Sharding: pure data parallel over the batch dim — 16 batches (80 beam-rows)
per NeuronCore, 8 cores.

Device algorithm (per core, 80 rows x 51200 padded vocab):
  A1  group-max: reduce_max over groups of 50 -> 1024 group maxes per row,
      computed in a [128 partitions, rows, 400] layout so the DVE scan uses
      all 128 partitions (one full pass over the data, overlapped with DMA).
  A2  PE-transpose the [128, 80, 8] group-max tensor into D [80 rows, 1024].
  A3  top-16 groups per row via max8 / max_index / match_replace / max8.
  A4  indirect-DMA gather of the 16 winning groups (50 vals each) from DRAM.
  A5  add the per-row running-score offset, then top-16 of the 800 gathered
      candidates (values + positions) via the max8 flow.
Host: decode positions -> vocab ids, apply the rank penalty, final
cross-beam top-10 over 50 candidates per batch row (tiny, O(bsz*50)).

The per-row top-k is exact: every group containing a top-10 element has a
group-max >= the 10th value, so the winner groups are a prefix of the groups
sorted by max; gathering 16 groups gives slack for rounding-tie edge cases.
"""

from contextlib import ExitStack

import numpy as np

import concourse.bacc as bacc
import concourse.bass as bass
import concourse.mybir as mybir
import concourse.tile as tile
from concourse import masks
from concourse.bass_utils import run_bass_kernel_spmd

# ---- geometry (hardcoded for this problem) ----
BSZ = 128
BEAM = 5
VOCAB = 50257
K = 10  # min(2*beam, beam*vocab-1)
DIVERSITY_RATE = 0.5

N_CORES = 8
B_PER_CORE = BSZ // N_CORES  # 16
R = B_PER_CORE * BEAM  # 80 rows per core
P = 128  # SBUF partitions
FPP = 416  # vocab elems per partition (padded)
VPAD = P * FPP  # 53248
GS = 32  # group size (128B runs: power-of-2 byte stride keeps the
# indirect-DMA's fp32 address arithmetic exact)
GPP = FPP // GS  # 13 groups per partition-chunk
NG = P * GPP  # 1664 groups per row
NSEL = 16  # groups gathered per row
GATH = NSEL * GS  # 800 candidates per row
RT = 16  # rows per DMA tile
NT = R // RT  # 5 tiles
NEG = -1.0e30

F32 = mybir.dt.float32
U32 = mybir.dt.uint32

_TRACE = False  # test.py flips this to profile
_LAST_RESULTS = None  # BassKernelResults of the last run (for test.py)


def build_nc(stop_after="full"):
    """stop_after in {"a1","a2","a3","a4","full"} — emit a truncated kernel
    (earlier stages only) for hardware bisection."""
    nc = bacc.Bacc(
        "TRN2", target_bir_lowering=False, debug=False, num_devices=N_CORES
    )
    lp = nc.dram_tensor("lp", [R, VPAD], F32, kind="ExternalInput")
    c_off = nc.dram_tensor("c_off", [R, 1], F32, kind="ExternalInput")
    o_valsA = nc.dram_tensor("valsA", [R, 8], F32, kind="ExternalOutput")
    o_valsB = nc.dram_tensor("valsB", [R, 8], F32, kind="ExternalOutput")
    o_posA = nc.dram_tensor("posA", [R, 8], U32, kind="ExternalOutput")
    o_posB = nc.dram_tensor("posB", [R, 8], U32, kind="ExternalOutput")
    o_gsel = nc.dram_tensor("gsel", [R, NSEL], U32, kind="ExternalOutput")

    stages = ["a1", "a2", "a3", "a4", "full"]
    lvl = stages.index(stop_after)

    def emit(tc, ctx):
        xpool = ctx.enter_context(tc.tile_pool(name="x", bufs=NT))
        spool = ctx.enter_context(tc.tile_pool(name="s", bufs=1))
        ppool = ctx.enter_context(tc.tile_pool(name="p", bufs=4, space="PSUM"))

        def bail(gsel_tile):
            # Truncated kernels only write gsel (zeros if not yet computed);
            # the other outputs come back as their donated zero buffers.
            if gsel_tile is not None:
                nc.gpsimd.dma_start(o_gsel.ap(), gsel_tile[:])
            else:
                ufill = spool.tile([R, NSEL], U32)
                nc.vector.memset(ufill[:], 0)
                nc.gpsimd.dma_start(o_gsel.ap(), ufill[:])

        ident = spool.tile([P, P], F32)
        masks.make_identity(nc, ident[:])

        gm = spool.tile([P, R, GPP], F32)  # group maxes, [p, r, g]
        # A1: stream row-tiles, group-max reduce
        for t in range(NT):
            x = xpool.tile([P, RT, FPP], F32, tag="x")
            src = lp.ap()[t * RT : (t + 1) * RT, :].rearrange(
                "r (p f) -> p r f", p=P
            )
            nc.sync.dma_start(x[:], src)
            nc.vector.reduce_max(
                gm[:, t * RT : (t + 1) * RT, :],
                x[:].rearrange("p r (g j) -> p r g j", j=GS),
                axis=mybir.AxisListType.X,
            )
        if lvl < 1:
            return bail(None)

        # A2: transpose [p, r, g] -> D[r, q] with q = p*GPP + g.
        # Rotating PSUM slots (each transpose gets a bank-aligned tile).
        D = spool.tile([R, NG], F32)
        dv = D[:].rearrange("r (p g) -> r p g", g=GPP)
        for g in range(GPP):
            pt = ppool.tile([R, P], F32, name=f"pt{g}", tag="pt")
            nc.tensor.transpose(pt[:], gm[:, :, g], ident[:])
            nc.scalar.copy(dv[:, :, g], pt[:])
        if lvl < 2:
            return bail(None)

        # A3: top-16 groups per row
        gsel = spool.tile([R, NSEL], U32)
        mA = spool.tile([R, 8], F32)
        nc.vector.max(out=mA[:], in_=D[:])
        nc.vector.max_index(out=gsel[:, 0:8], in_max=mA[:], in_values=D[:])
        D2 = spool.tile([R, NG], F32)
        nc.vector.match_replace(
            out=D2[:], in_to_replace=mA[:], in_values=D[:], imm_value=NEG
        )
        mB = spool.tile([R, 8], F32)
        nc.vector.max(out=mB[:], in_=D2[:])
        nc.vector.max_index(out=gsel[:, 8:16], in_max=mB[:], in_values=D2[:])
        if lvl < 3:
            return bail(gsel)

        # A4: gather the winning groups from DRAM
        rowbase = spool.tile([R, NSEL], U32)
        nc.gpsimd.iota(
            rowbase[:], pattern=[[0, NSEL]], base=0, channel_multiplier=NG
        )
        gidx = spool.tile([R, NSEL], U32)
        nc.vector.tensor_tensor(
            out=gidx[:], in0=gsel[:], in1=rowbase[:], op=mybir.AluOpType.add
        )
        # One indirect DMA per slot: offset AP [R, 1] (one offset per
        # partition) is the pattern the SWDGE ucode supports; 128B runs
        # keep its fp32 address math exact.
        gt = spool.tile([R, NSEL, GS], F32)
        gsrc = lp.ap().rearrange("r (q j) -> (r q) j", j=GS)
        for s in range(NSEL):
            nc.gpsimd.indirect_dma_start(
                out=gt[:, s, :],
                out_offset=None,
                in_=gsrc,
                in_offset=bass.IndirectOffsetOnAxis(
                    ap=gidx[:, s : s + 1], axis=0
                ),
            )
        if lvl < 4:
            return bail(gsel)

        # A5: add running-score offset, top-16 of the 800 candidates
        cof = spool.tile([R, 1], F32)
        nc.gpsimd.dma_start(cof[:], c_off.ap())
        gv = spool.tile([R, GATH], F32)
        nc.vector.tensor_scalar(
            out=gv[:],
            in0=gt[:].rearrange("r s j -> r (s j)"),
            scalar1=cof[:, 0:1],
            scalar2=None,
            op0=mybir.AluOpType.add,
        )
        vA = spool.tile([R, 8], F32)
        pA = spool.tile([R, 8], U32)
        nc.vector.max(out=vA[:], in_=gv[:])
        nc.vector.max_index(out=pA[:], in_max=vA[:], in_values=gv[:])
        gv2 = spool.tile([R, GATH], F32)
        nc.vector.match_replace(
            out=gv2[:], in_to_replace=vA[:], in_values=gv[:], imm_value=NEG
        )
        vB = spool.tile([R, 8], F32)
        pB = spool.tile([R, 8], U32)
        nc.vector.max(out=vB[:], in_=gv2[:])
        nc.vector.max_index(out=pB[:], in_max=vB[:], in_values=gv2[:])

        nc.gpsimd.dma_start(o_valsA.ap(), vA[:])
        nc.gpsimd.dma_start(o_valsB.ap(), vB[:])
        nc.gpsimd.dma_start(o_posA.ap(), pA[:])
        nc.gpsimd.dma_start(o_posB.ap(), pB[:])
        nc.gpsimd.dma_start(o_gsel.ap(), gsel[:])

    with tile.TileContext(nc) as tc, ExitStack() as ctx:
        emit(tc, ctx)

    nc.compile()
    return nc


_NC = None


def _get_nc():
    global _NC
    if _NC is None:
        _NC = build_nc()
    return _NC


def make_in_maps(lprobs, scores, step):
    """Pad + shard the host inputs into per-core input maps."""
    lprobs = np.asarray(lprobs, dtype=np.float32)
    scores = np.asarray(scores, dtype=np.float32)
    step = int(step)
    pad = np.full((BSZ, BEAM, VPAD - VOCAB), NEG, dtype=np.float32)
    lp_pad = np.concatenate([lprobs, pad], axis=-1)  # [128, 5, 51200]
    c_off = scores[:, :, step - 1]  # [128, 5]
    in_maps = []
    for c in range(N_CORES):
        b0, b1 = c * B_PER_CORE, (c + 1) * B_PER_CORE
        in_maps.append(
            {
                "lp": np.ascontiguousarray(lp_pad[b0:b1].reshape(R, VPAD)),
                "c_off": np.ascontiguousarray(c_off[b0:b1].reshape(R, 1)),
            }
        )
    return in_maps


def postprocess(results):
    """Per-core device outputs -> full final (scores, indices, beams)."""
    vals16 = np.concatenate(
        [np.concatenate([r["valsA"], r["valsB"]], axis=1) for r in results],
        axis=0,
    )  # [640, 16] f32, descending per row
    pos16 = np.concatenate(
        [np.concatenate([r["posA"], r["posB"]], axis=1) for r in results],
        axis=0,
    ).astype(np.int64)  # positions in the 800-candidate buffer
    gsel = np.concatenate([r["gsel"] for r in results], axis=0).astype(
        np.int64
    )  # [640, 16] group ids (q), vocab start = 50*q

    nrows = BSZ * BEAM
    top_vals = vals16[:, :K]  # [640, 10]
    top_pos = pos16[:, :K]
    q = np.take_along_axis(gsel, top_pos // GS, axis=1)  # [640, 10]
    top_vocab = q * GS + top_pos % GS  # [640, 10]

    s = top_vals.reshape(BSZ, BEAM, K) - (
        np.arange(1, K + 1, dtype=np.float32) * np.float32(DIVERSITY_RATE)
    )
    indices = top_vocab.reshape(BSZ, BEAM * K)
    s50 = s.reshape(BSZ, BEAM * K)

    flat_pos = np.argsort(-s50, axis=1, kind="stable")[:, :K]
    final_scores = np.take_along_axis(s50, flat_pos, axis=1)
    final_indices = np.take_along_axis(indices, flat_pos, axis=1).astype(
        np.int32
    )
    final_beams = (flat_pos // K).astype(np.int32)
    return final_scores, final_indices, final_beams


def kernel(lprobs, scores, step):
    global _LAST_RESULTS
    nc = _get_nc()
    in_maps = make_in_maps(lprobs, scores, step)
    res = run_bass_kernel_spmd(
        nc, in_maps, core_ids=list(range(N_CORES)), trace=_TRACE
    )
    _LAST_RESULTS = res
    return postprocess(res.results)


# revision 12
# speedup vs baseline: 1.5688x; 1.5688x over previous
"""Trainium2 Bass kernel for DiverseSiblingsSearch (per-beam top-k + sibling
penalty + cross-beam top-k).

Contract: kernel(**inputs) takes the FULL inputs (lprobs [128,5,50257] f32,
scores [128,5,10] f32, step scalar) and returns the FULL outputs
(final_scores [128,10] f32, final_indices [128,10] i32, final_beams [128,10] i32).

Sharding: pure data parallel over the batch dim — 16 batches (80 beam-rows)
per NeuronCore, 8 cores.

Device algorithm (per core, 80 rows x 53248 padded vocab; the full
25.7M-element scan and the top-k selection):
  A1  group-max: reduce_max over groups of 32 -> 1664 group maxes per row,
      computed in a [128 partitions, rows, 416] layout so the DVE scan uses
      all 128 partitions; DMA tiles of 8 rows multi-buffered so the scan
      hides under the HBM stream.
  A2  PE-transpose the [128, 80, 13] group-max tensor into D [80 rows, 1664]
      (group q = p*13 + g covers vocab [32q, 32q+32)), then reduce runs of 4
      into super-group maxes sgm [80, 416] (super-group covers 128 vocab).
  A3  top-16 super-groups per row via max8 / max_index / match_replace /
      max8 / max_index -> gsel [80, 16].
Host: gather the 16 winning 128-wide vocab spans per row from lprobs
(guaranteed to contain the row's top-10: any group holding a top-10 element
has group-max >= the 10th value, so winner groups are a prefix of groups
sorted by max — at most 10 of them), add the running score, exact top-10 per
row, rank penalty, cross-beam top-10 over 50, final gather. O(bsz*beam*2k)
numpy work.
"""

from contextlib import ExitStack

import numpy as np

import concourse.bacc as bacc
import concourse.mybir as mybir
import concourse.tile as tile
from concourse import masks
from concourse.bass_utils import run_bass_kernel_spmd

# ---- geometry (hardcoded for this problem) ----
BSZ = 128
BEAM = 5
VOCAB = 50257
K = 10  # min(2*beam, beam*vocab-1)
DIVERSITY_RATE = 0.5

N_CORES = 8
B_PER_CORE = BSZ // N_CORES  # 16
R = B_PER_CORE * BEAM  # 80 rows per core
P = 128  # SBUF partitions
FPP = 416  # vocab elems per partition (padded)
VPAD = P * FPP  # 53248
GS = 32  # group size
GPP = FPP // GS  # 13 groups per partition-chunk
NG = P * GPP  # 1664 groups per row
SGF = 4  # groups per super-group
NSG = NG // SGF  # 416 super-groups per row
SGS = GS * SGF  # 128 vocab per super-group
NSEL = 16  # super-groups selected per row
RT = 8  # rows per DMA tile
NT = R // RT  # 10 tiles
NEG = -1.0e30

F32 = mybir.dt.float32
U32 = mybir.dt.uint32

_TRACE = False  # test.py flips this to profile
_LAST_RESULTS = None  # BassKernelResults of the last run (for test.py)


def build_nc():
    nc = bacc.Bacc(
        "TRN2", target_bir_lowering=False, debug=False, num_devices=N_CORES
    )
    lp = nc.dram_tensor("lp", [R, VPAD], F32, kind="ExternalInput")
    o_gsel = nc.dram_tensor("gsel", [R, NSEL], U32, kind="ExternalOutput")

    def emit(tc, ctx):
        xpool = ctx.enter_context(tc.tile_pool(name="x", bufs=NT))
        spool = ctx.enter_context(tc.tile_pool(name="s", bufs=1))
        ppool = ctx.enter_context(tc.tile_pool(name="p", bufs=4, space="PSUM"))

        ident = spool.tile([P, P], F32)
        masks.make_identity(nc, ident[:])

        gm = spool.tile([P, R, GPP], F32)  # group maxes, [p, r, g]
        # A1: stream row-tiles, group-max reduce
        for t in range(NT):
            x = xpool.tile([P, RT, FPP], F32, tag="x")
            src = lp.ap()[t * RT : (t + 1) * RT, :].rearrange(
                "r (p f) -> p r f", p=P
            )
            nc.sync.dma_start(x[:], src)
            nc.vector.reduce_max(
                gm[:, t * RT : (t + 1) * RT, :],
                x[:].rearrange("p r (g j) -> p r g j", j=GS),
                axis=mybir.AxisListType.X,
            )

        # A2: transpose [p, r, g] -> D[r, q] with q = p*GPP + g.
        # Rotating PSUM slots (each transpose gets a bank-aligned tile).
        D = spool.tile([R, NG], F32)
        dv = D[:].rearrange("r (p g) -> r p g", g=GPP)
        for g in range(GPP):
            pt = ppool.tile([R, P], F32, name=f"pt{g}", tag="pt")
            nc.tensor.transpose(pt[:], gm[:, :, g], ident[:])
            nc.scalar.copy(dv[:, :, g], pt[:])
        # super-group maxes (128 contiguous vocab each: vocab = 32q)
        sgm = spool.tile([R, NSG], F32)
        nc.vector.reduce_max(
            sgm[:],
            D[:].rearrange("r (s f) -> r s f", f=SGF),
            axis=mybir.AxisListType.X,
        )

        # A3: top-16 super-groups per row
        gsel = spool.tile([R, NSEL], U32)
        mA = spool.tile([R, 8], F32)
        nc.vector.max(out=mA[:], in_=sgm[:])
        nc.vector.max_index(out=gsel[:, 0:8], in_max=mA[:], in_values=sgm[:])
        sg2 = spool.tile([R, NSG], F32)
        nc.vector.match_replace(
            out=sg2[:], in_to_replace=mA[:], in_values=sgm[:], imm_value=NEG
        )
        mB = spool.tile([R, 8], F32)
        nc.vector.max(out=mB[:], in_=sg2[:])
        nc.vector.max_index(out=gsel[:, 8:16], in_max=mB[:], in_values=sg2[:])

        nc.gpsimd.dma_start(o_gsel.ap(), gsel[:])

    with tile.TileContext(nc) as tc, ExitStack() as ctx:
        emit(tc, ctx)

    nc.compile()
    return nc


_NC = None


def _get_nc():
    global _NC
    if _NC is None:
        _NC = build_nc()
    return _NC


def make_in_maps(lprobs):
    """Pad + shard lprobs into per-core input maps."""
    pad = np.full((BSZ, BEAM, VPAD - VOCAB), NEG, dtype=np.float32)
    lp_pad = np.concatenate([lprobs, pad], axis=-1)  # [128, 5, 53248]
    in_maps = []
    for c in range(N_CORES):
        b0, b1 = c * B_PER_CORE, (c + 1) * B_PER_CORE
        in_maps.append(
            {"lp": np.ascontiguousarray(lp_pad[b0:b1].reshape(R, VPAD))}
        )
    return in_maps


def postprocess(results, lprobs, scores, step):
    """Device super-group selection -> exact full outputs on host.

    The device guarantees each row's top-10 lives inside its 16 selected
    128-wide vocab spans; everything past this point is O(bsz*beam*2k).
    """
    nrows = BSZ * BEAM
    gsel = np.concatenate([r["gsel"] for r in results], axis=0).astype(
        np.int64
    )  # [640, 16] super-group ids; vocab span = [128*sg, 128*sg+128)

    lpr = lprobs.reshape(nrows, VOCAB)
    c = scores.reshape(nrows, -1)[:, step - 1].astype(np.float32)

    # gather candidate spans (clip into the real vocab; padding never wins)
    span = gsel[:, :, None] * SGS + np.arange(SGS)[None, None, :]
    span_c = np.minimum(span, VOCAB - 1).reshape(nrows, -1)
    oob = (span >= VOCAB).reshape(nrows, -1)
    cand = np.take_along_axis(lpr, span_c, axis=1)
    cand = np.where(oob, np.float32(NEG), cand)
    cand = cand + c[:, None]  # running-score offset, f32 like the reference

    # exact per-row top-10 (value desc, ties -> lower vocab id, like lax.top_k)
    vocab_ids = np.where(oob, VOCAB, span.reshape(nrows, -1))
    order = np.lexsort((vocab_ids, -cand), axis=1)[:, :K]
    top_vals = np.take_along_axis(cand, order, axis=1)  # [640, 10]
    top_vocab = np.take_along_axis(vocab_ids, order, axis=1)

    s = top_vals.reshape(BSZ, BEAM, K) - (
        np.arange(1, K + 1, dtype=np.float32) * np.float32(DIVERSITY_RATE)
    )
    s50 = s.reshape(BSZ, BEAM * K)
    indices = top_vocab.reshape(BSZ, BEAM * K)

    flat_pos = np.argsort(-s50, axis=1, kind="stable")[:, :K]
    final_scores = np.take_along_axis(s50, flat_pos, axis=1)
    final_indices = np.take_along_axis(indices, flat_pos, axis=1).astype(
        np.int32
    )
    final_beams = (flat_pos // K).astype(np.int32)
    return final_scores, final_indices, final_beams


def kernel(lprobs, scores, step):
    global _LAST_RESULTS
    lprobs = np.asarray(lprobs, dtype=np.float32)
    scores = np.asarray(scores, dtype=np.float32)
    step = int(step)
    nc = _get_nc()
    in_maps = make_in_maps(lprobs)
    res = run_bass_kernel_spmd(
        nc, in_maps, core_ids=list(range(N_CORES)), trace=_TRACE
    )
    _LAST_RESULTS = res
    return postprocess(res.results, lprobs, scores, step)


# revision 13
# speedup vs baseline: 1.6829x; 1.0727x over previous
"""Trainium2 Bass kernel for DiverseSiblingsSearch (per-beam top-k + sibling
penalty + cross-beam top-k).

Contract: kernel(**inputs) takes the FULL inputs (lprobs [128,5,50257] f32,
scores [128,5,10] f32, step scalar) and returns the FULL outputs
(final_scores [128,10] f32, final_indices [128,10] i32, final_beams [128,10] i32).

Sharding: pure data parallel over the batch dim — 16 batches (80 beam-rows)
per NeuronCore, 8 cores.

Device algorithm (per core, 80 rows x 51200 padded vocab; the full
25.7M-element scan and the top-k selection):
  A1  group-max: reduce_max over groups of 50 -> 1024 group maxes per row,
      computed in a [128 partitions, rows, 400] layout so the DVE scan uses
      all 128 partitions; DMA tiles of 4 rows multi-buffered so the scan
      hides under the HBM stream.
  A2  PE-transpose the [128, 80, 8] group-max tensor into D [80 rows, 1024]
      (group q = p*8 + g covers vocab [50q, 50q+50)), then reduce runs of 4
      into super-group maxes sgm [80, 256] (super-group covers 200 vocab).
  A3  top-16 super-groups per row via max8 / max_index / match_replace /
      max8 / max_index -> gsel [80, 16].
Host: gather the 16 winning 200-wide vocab spans per row from lprobs
(guaranteed to contain the row's top-10: any group holding a top-10 element
has group-max >= the 10th value, so winner groups are a prefix of groups
sorted by max — at most 10 of them), add the running score, exact top-10 per
row, rank penalty, cross-beam top-10 over 50, final gather. O(bsz*beam*2k)
numpy work.
"""

from contextlib import ExitStack

import numpy as np

import concourse.bacc as bacc
import concourse.mybir as mybir
import concourse.tile as tile
from concourse import masks
from concourse.bass_utils import run_bass_kernel_spmd

# ---- geometry (hardcoded for this problem) ----
BSZ = 128
BEAM = 5
VOCAB = 50257
K = 10  # min(2*beam, beam*vocab-1)
DIVERSITY_RATE = 0.5

N_CORES = 8
B_PER_CORE = BSZ // N_CORES  # 16
R = B_PER_CORE * BEAM  # 80 rows per core
P = 128  # SBUF partitions
FPP = 400  # vocab elems per partition (padded)
VPAD = P * FPP  # 51200
GS = 50  # group size
GPP = FPP // GS  # 8 groups per partition-chunk
NG = P * GPP  # 1024 groups per row
SGF = 4  # groups per super-group
NSG = NG // SGF  # 256 super-groups per row
SGS = GS * SGF  # 200 vocab per super-group
NSEL = 16  # super-groups selected per row
RT = 4  # rows per DMA tile
NT = R // RT  # 20 tiles
NEG = -1.0e30

F32 = mybir.dt.float32
U32 = mybir.dt.uint32

_TRACE = False  # test.py flips this to profile
_LAST_RESULTS = None  # BassKernelResults of the last run (for test.py)


def build_nc():
    nc = bacc.Bacc(
        "TRN2", target_bir_lowering=False, debug=False, num_devices=N_CORES
    )
    lp = nc.dram_tensor("lp", [R, VPAD], F32, kind="ExternalInput")
    o_gsel = nc.dram_tensor("gsel", [R, NSEL], U32, kind="ExternalOutput")

    def emit(tc, ctx):
        xpool = ctx.enter_context(tc.tile_pool(name="x", bufs=NT))
        spool = ctx.enter_context(tc.tile_pool(name="s", bufs=1))
        ppool = ctx.enter_context(tc.tile_pool(name="p", bufs=4, space="PSUM"))

        ident = spool.tile([P, P], F32)
        masks.make_identity(nc, ident[:])

        gm = spool.tile([P, R, GPP], F32)  # group maxes, [p, r, g]
        # A1: stream row-tiles, group-max reduce
        for t in range(NT):
            x = xpool.tile([P, RT, FPP], F32, tag="x")
            src = lp.ap()[t * RT : (t + 1) * RT, :].rearrange(
                "r (p f) -> p r f", p=P
            )
            nc.sync.dma_start(x[:], src)
            nc.vector.reduce_max(
                gm[:, t * RT : (t + 1) * RT, :],
                x[:].rearrange("p r (g j) -> p r g j", j=GS),
                axis=mybir.AxisListType.X,
            )

        # A2: transpose [p, r, g] -> D[r, q] with q = p*GPP + g.
        # Rotating PSUM slots (each transpose gets a bank-aligned tile).
        D = spool.tile([R, NG], F32)
        dv = D[:].rearrange("r (p g) -> r p g", g=GPP)
        for g in range(GPP):
            pt = ppool.tile([R, P], F32, name=f"pt{g}", tag="pt")
            nc.tensor.transpose(pt[:], gm[:, :, g], ident[:])
            nc.scalar.copy(dv[:, :, g], pt[:])
        # super-group maxes (128 contiguous vocab each: vocab = 32q)
        sgm = spool.tile([R, NSG], F32)
        nc.vector.reduce_max(
            sgm[:],
            D[:].rearrange("r (s f) -> r s f", f=SGF),
            axis=mybir.AxisListType.X,
        )

        # A3: top-16 super-groups per row
        gsel = spool.tile([R, NSEL], U32)
        mA = spool.tile([R, 8], F32)
        nc.vector.max(out=mA[:], in_=sgm[:])
        nc.vector.max_index(out=gsel[:, 0:8], in_max=mA[:], in_values=sgm[:])
        sg2 = spool.tile([R, NSG], F32)
        nc.vector.match_replace(
            out=sg2[:], in_to_replace=mA[:], in_values=sgm[:], imm_value=NEG
        )
        mB = spool.tile([R, 8], F32)
        nc.vector.max(out=mB[:], in_=sg2[:])
        nc.vector.max_index(out=gsel[:, 8:16], in_max=mB[:], in_values=sg2[:])

        nc.sync.dma_start(o_gsel.ap(), gsel[:])

    with tile.TileContext(nc) as tc, ExitStack() as ctx:
        emit(tc, ctx)

    nc.compile()
    return nc


_NC = None


def _get_nc():
    global _NC
    if _NC is None:
        _NC = build_nc()
    return _NC


def make_in_maps(lprobs):
    """Pad + shard lprobs into per-core input maps."""
    pad = np.full((BSZ, BEAM, VPAD - VOCAB), NEG, dtype=np.float32)
    lp_pad = np.concatenate([lprobs, pad], axis=-1)  # [128, 5, 51200]
    in_maps = []
    for c in range(N_CORES):
        b0, b1 = c * B_PER_CORE, (c + 1) * B_PER_CORE
        in_maps.append(
            {"lp": np.ascontiguousarray(lp_pad[b0:b1].reshape(R, VPAD))}
        )
    return in_maps


def postprocess(results, lprobs, scores, step):
    """Device super-group selection -> exact full outputs on host.

    The device guarantees each row's top-10 lives inside its 16 selected
    128-wide vocab spans; everything past this point is O(bsz*beam*2k).
    """
    nrows = BSZ * BEAM
    gsel = np.concatenate([r["gsel"] for r in results], axis=0).astype(
        np.int64
    )  # [640, 16] super-group ids; vocab span = [200*sg, 200*sg+200)

    lpr = lprobs.reshape(nrows, VOCAB)
    c = scores.reshape(nrows, -1)[:, step - 1].astype(np.float32)

    # gather candidate spans (clip into the real vocab; padding never wins)
    span = gsel[:, :, None] * SGS + np.arange(SGS)[None, None, :]
    span_c = np.minimum(span, VOCAB - 1).reshape(nrows, -1)
    oob = (span >= VOCAB).reshape(nrows, -1)
    cand = np.take_along_axis(lpr, span_c, axis=1)
    cand = np.where(oob, np.float32(NEG), cand)
    cand = cand + c[:, None]  # running-score offset, f32 like the reference

    # exact per-row top-10 (value desc, ties -> lower vocab id, like lax.top_k)
    vocab_ids = np.where(oob, VOCAB, span.reshape(nrows, -1))
    order = np.lexsort((vocab_ids, -cand), axis=1)[:, :K]
    top_vals = np.take_along_axis(cand, order, axis=1)  # [640, 10]
    top_vocab = np.take_along_axis(vocab_ids, order, axis=1)

    s = top_vals.reshape(BSZ, BEAM, K) - (
        np.arange(1, K + 1, dtype=np.float32) * np.float32(DIVERSITY_RATE)
    )
    s50 = s.reshape(BSZ, BEAM * K)
    indices = top_vocab.reshape(BSZ, BEAM * K)

    flat_pos = np.argsort(-s50, axis=1, kind="stable")[:, :K]
    final_scores = np.take_along_axis(s50, flat_pos, axis=1)
    final_indices = np.take_along_axis(indices, flat_pos, axis=1).astype(
        np.int32
    )
    final_beams = (flat_pos // K).astype(np.int32)
    return final_scores, final_indices, final_beams


def kernel(lprobs, scores, step):
    global _LAST_RESULTS
    lprobs = np.asarray(lprobs, dtype=np.float32)
    scores = np.asarray(scores, dtype=np.float32)
    step = int(step)
    nc = _get_nc()
    in_maps = make_in_maps(lprobs)
    res = run_bass_kernel_spmd(
        nc, in_maps, core_ids=list(range(N_CORES)), trace=_TRACE
    )
    _LAST_RESULTS = res
    return postprocess(res.results, lprobs, scores, step)
